# revision 2
# baseline (speedup 1.0000x reference)
"""Trainium2 Bass kernel for masked multi-head attention with a rope-like
positional transform (nn_Attention_43937515438607) — v3.

Architecture (per core, 2 batches):
  - Q,K projected in TRANSPOSED [d, tok] layout (lhsT = W chunk, rhs = x^T);
    V in natural layout with a ones-column for the softmax denominator.
  - rope: bias added in the PSUM->SBUF staging copy (tensor_scalar with a
    per-partition bias AP); pair-swap via one PE matmul against a constant
    128x128 pair-permutation; two fp16 multiplies + add on VectorE.
  - scores: two K=64 matmuls per kv tile into disjoint PE row groups
    (auto tile_position from base partitions 0/64) -> run CONCURRENTLY.
  - softmax: exp on ScalarE straight out of PSUM (no max subtraction,
    validated range), alpha in bf16; {0,1} mask multiply on VectorE.
  - AV: V_ext (64 v rows + ones row) as stationary, alpha moving,
    accumulated over kv tiles; output normalized after a PE transpose.
  - emission is software-pipelined at CHUNK granularity: projection and
    output-normalization chunks for future units are fed one-at-a-time
    into the attention kt-loop, so no engine FIFO gets a long run of
    potentially-blocked instructions (head-of-line blocking).

Measured per-op HW costs that shaped this: exp[128,1024] 1218ns;
TT[128,512]16b 241ns; PSUM-source DVE ops ~630-980ns; S-pair 225ns
(concurrent); AV mm 222ns; GpSimd TT 1203ns + it locks the shared SBUF
port pair against DVE tensor_tensor ops, so GpSimd gets no elementwise
work; scalar_tensor_tensor and stream_shuffle are ~776ns (avoided).
"""

import sys

try:
    import concourse  # noqa: F401
except ImportError:  # pragma: no cover
    sys.path.insert(0, "/opt/trn_rl_repo")

import numpy as np
import ml_dtypes

from concourse import bass, mybir, tile, bacc
from concourse.bass_utils import run_bass_kernel_spmd
from concourse.masks import make_identity

B, T, C = 16, 1024, 512
NH = 8
D = C // NH
TP = float((2.0 * D) ** 0.5)
N_CORES = 8
BPC = B // N_CORES            # batches per core = 2
TOK = BPC * T                 # tokens per core  = 2048
NTT = TOK // 128              # token tiles per core = 16
NTB = T // 128                # token tiles per batch = 8
NHP = NH // 2                 # head pairs = 4
QC = 512                      # q chunk (PSUM bank) per attention unit
NQC = T // QC                 # q chunks per batch = 2
NG = TOK // 512               # 512-token groups per core = 4
VW = 66 * NH + 32             # V_ext row width = 560

F32 = mybir.dt.float32
F32R = mybir.dt.float32r
F16 = mybir.dt.float16
BF16 = mybir.dt.bfloat16
MULT = mybir.AluOpType.mult
ADD = mybir.AluOpType.add


def build_nc(niter=1):
    nc = bacc.Bacc("TRN2", target_bir_lowering=False, debug=False)

    xT_d = nc.dram_tensor("xT", [C, TOK], F16, kind="ExternalInput")
    wT_d = nc.dram_tensor("wT", [C, 3 * C], F16, kind="ExternalInput")
    brow_d = nc.dram_tensor("brow_v", [1, C], F16, kind="ExternalInput")
    ones_d = nc.dram_tensor("ones_row", [1, 128], F16, kind="ExternalInput")
    swp_d = nc.dram_tensor("swp", [128, 128], F16, kind="ExternalInput")
    bqk_d = nc.dram_tensor("bias_qk", [128, 8], F32, kind="ExternalInput")
    rope_d = nc.dram_tensor("ropeT", [4, 128, T], F16, kind="ExternalInput")
    nmT_d = nc.dram_tensor("nmT", [BPC, T, T], BF16, kind="ExternalInput")
    y_d = nc.dram_tensor("y", [TOK, C], F32, kind="ExternalOutput")

    with tile.TileContext(nc) as tc:
        import contextlib
        loop_cm = (tc.For_i(0, niter, 1, staggered_reset=True,
                            hint_engines=(mybir.EngineType.PE,
                                          mybir.EngineType.DVE,
                                          mybir.EngineType.Activation,
                                          mybir.EngineType.SP))
                   if niter > 1 else contextlib.nullcontext())
        ctx = contextlib.ExitStack()
        with loop_cm, ctx:
            persist = ctx.enter_context(tc.tile_pool(name="persist", bufs=1))
            V_sb = persist.tile([128, NTT, VW], BF16)
            QT = [persist.tile([128, NHP, T], F16, tag=f"QT{b}", name=f"QT{b}")
                  for b in range(BPC)]
            KT = [persist.tile([128, NHP, T], F16, tag=f"KT{b}", name=f"KT{b}")
                  for b in range(BPC)]
            OT = [persist.tile([96, NH, T], F32R, tag=f"OT{b}", name=f"OT{b}")
                  for b in range(BPC)]
            mT = [persist.tile([128, NTB, T], BF16, tag=f"mT{b}", name=f"mT{b}")
                  for b in range(BPC)]
            wt = persist.tile([128, 4, 3 * C], F16)
            xg = persist.tile([128, NG, 4, 512], F16)     # [ci, group, ko, tok]
            rp = persist.tile([128, 4, T], F16)           # Aq, Bq, Ak, Bk
            swp = persist.tile([128, 128], F16)
            bqk = persist.tile([128, 8], F32)
            ones1 = persist.tile([1, 128], F16)
            brow = persist.tile([1, C], F16)
            id_tmp = persist.tile([128, 128], F32)
            id_f32 = persist.tile([128, 128], F32R)

            make_identity(nc, id_tmp[:])
            nc.vector.tensor_copy(id_f32[:], id_tmp[:])
            nc.gpsimd.memset(V_sb[:], 0.0)
            nc.vector.memset(V_sb[:, :, 64::66], 1.0)

            # ---------- input DMAs ----------
            for ko in range(4):
                nc.sync.dma_start(wt[:, ko, :], wT_d[ko * 128:(ko + 1) * 128, :])
            nc.sync.dma_start(ones1[:], ones_d[:])
            nc.sync.dma_start(brow[:], brow_d[:])
            nc.sync.dma_start(swp[:], swp_d[:])
            nc.sync.dma_start(bqk[:], bqk_d[:])
            nc.sync.dma_start(rp[:], rope_d.rearrange("f p t -> p f t"))
            for g in range(NG):
                nc.sync.dma_start(
                    xg[:, g],
                    xT_d[:, g * 512:(g + 1) * 512].rearrange(
                        "(ko p) t -> p ko t", p=128))
            for b in range(BPC):
                for kg in range(4):
                    nc.sync.dma_start(
                        mT[b][:, kg * 2:(kg + 1) * 2, :],
                        nmT_d[b][kg * 256:(kg + 1) * 256, :].rearrange(
                            "(kt p) q -> p kt q", p=128))

            pp = ctx.enter_context(tc.tile_pool(name="pp", bufs=2, space="PSUM"))
            s_ps = ctx.enter_context(tc.tile_pool(name="s_ps", bufs=2, space="PSUM"))
            o_ps = ctx.enter_context(tc.tile_pool(name="o_ps", bufs=1, space="PSUM"))
            qsb_pool = ctx.enter_context(tc.tile_pool(name="qsb", bufs=2))
            t_pool = ctx.enter_context(tc.tile_pool(name="tpl", bufs=2))
            alpha_pool = ctx.enter_context(tc.tile_pool(name="alpha", bufs=4))
            fin_sb = ctx.enter_context(tc.tile_pool(name="fin_sb", bufs=3))

            def v_chunk(b, g01, t):
                g = 2 * b + g01
                tt = 4 * g + t
                ps = pp.tile([128, 512], F32, tag="pp", name="vps")
                for ko in range(4):
                    nc.tensor.matmul(
                        ps[:], xg[:, g, ko, t * 128:(t + 1) * 128],
                        wt[:, ko, 2 * C:3 * C], start=(ko == 0), stop=False)
                nc.tensor.matmul(ps[:], ones1[:], brow[:],
                                 start=False, stop=True)
                vdst = V_sb[:, tt, :528].rearrange(
                    "p (h e) -> p h e", h=NH)[:, :, :D]
                nc.vector.tensor_copy(
                    vdst, ps[:].rearrange("p (h d) -> p h d", h=NH))

            def qk_chunk(b, hp, fc, g01):
                col0 = fc * C + hp * 128
                dstt = (QT if fc == 0 else KT)[b]
                g = 2 * b + g01
                tsl = slice(g01 * 512, (g01 + 1) * 512)
                ps = pp.tile([128, 512], F32, tag="pp", name="qkps")
                for ko in range(4):
                    nc.tensor.matmul(
                        ps[:], wt[:, ko, col0:col0 + 128],
                        xg[:, g, ko, :], start=(ko == 0), stop=(ko == 3))
                qsb = qsb_pool.tile([128, 512], F16, tag="qsb")
                nc.vector.tensor_scalar(
                    qsb[:], ps[:], bqk[:, 4 * fc + hp:4 * fc + hp + 1],
                    None, ADD)
                sw = pp.tile([128, 512], F32, tag="pp", name="swps")
                nc.tensor.matmul(sw[:], swp[:], qsb[:], start=True, stop=True)
                t1 = t_pool.tile([128, 512], F16, tag="t1")
                nc.vector.tensor_tensor(t1[:], qsb[:], rp[:, 2 * fc, tsl], MULT)
                t2 = t_pool.tile([128, 512], F16, tag="t2")
                nc.vector.tensor_tensor(t2[:], sw[:], rp[:, 2 * fc + 1, tsl],
                                        MULT)
                nc.vector.tensor_tensor(dstt[:, hp, tsl], t1[:], t2[:], ADD)

            def fin_chunk(b, half, qt):
                out_sb = fin_sb.tile([128, C // 2], F32, tag="out")
                fp = pp.tile([128, 4 * 96], F32R, tag="pp", name="fin")
                for hh in range(4):
                    h = half * 4 + hh
                    nc.tensor.matmul(
                        fp[:, hh * 96:(hh + 1) * 96],
                        OT[b][:, h, qt * 128:(qt + 1) * 128],
                        id_f32[0:96, 0:96], is_transpose=True)
                rc = fin_sb.tile([128, 4], F32, tag="rc")
                nc.vector.reciprocal(rc[:], fp[:, 64::96])
                nc.vector.tensor_tensor(
                    out_sb[:].rearrange("p (h d) -> p h d", h=4),
                    fp[:].rearrange("p (h e) -> p h e", e=96)[:, :, :D],
                    rc[:][:, :, None].to_broadcast([128, 4, D]), MULT)
                row = b * T + qt * 128
                nc.sync.dma_start(
                    y_d[row:row + 128, half * 256:(half + 1) * 256], out_sb[:])

            def attention(b, hp, feed):
                hA, hB = 2 * hp, 2 * hp + 1
                for qc in range(NQC):
                    qsl = slice(qc * QC, (qc + 1) * QC)
                    oo = o_ps.tile([96, 2 * QC], F32, tag="oo")
                    oA, oB = oo[:, 0:QC], oo[:, QC:2 * QC]

                    def emit_av(al, kt):
                        vbase = b * NTB + kt
                        nc.tensor.matmul(
                            oA, V_sb[:, vbase, hA * 66:hA * 66 + 96],
                            al[:, 0:QC],
                            start=(kt == 0), stop=(kt == NTB - 1))
                        nc.tensor.matmul(
                            oB, V_sb[:, vbase, hB * 66:hB * 66 + 96],
                            al[:, QC:2 * QC],
                            start=(kt == 0), stop=(kt == NTB - 1))

                    pend = []
                    for kt in range(NTB):
                        sp = s_ps.tile([128, 2 * QC], F32, tag="s")
                        nc.tensor.matmul(
                            sp[:, 0:QC],
                            KT[b][0:64, hp, kt * 128:(kt + 1) * 128],
                            QT[b][0:64, hp, qsl], start=True, stop=True)
                        nc.tensor.matmul(
                            sp[:, QC:2 * QC],
                            KT[b][64:128, hp, kt * 128:(kt + 1) * 128],
                            QT[b][64:128, hp, qsl], start=True, stop=True)
                        al = alpha_pool.tile([128, 2 * QC], BF16, tag="al")
                        nc.scalar.activation(
                            al[:], sp[:],
                            mybir.ActivationFunctionType.Exp, scale=1.0 / TP)
                        msl = mT[b][:, kt, qsl]
                        nc.vector.tensor_tensor(
                            al[:, 0:QC], al[:, 0:QC], msl, MULT)
                        nc.vector.tensor_tensor(
                            al[:, QC:2 * QC], al[:, QC:2 * QC], msl, MULT)
                        pend.append((al, kt))
                        if len(pend) > 2:
                            emit_av(*pend.pop(0))
                        if feed:
                            feed.pop(0)()
                    for p_ in pend:
                        emit_av(*p_)
                    nc.vector.tensor_copy(
                        OT[b][:, hA:hB + 1, qsl], oo[:].rearrange(
                            "p (h q) -> p h q", h=2))

            def qk_chunks(b, hp):
                return [lambda b=b, hp=hp, fc=fc, g01=g01:
                        qk_chunk(b, hp, fc, g01)
                        for fc in range(2) for g01 in range(2)]

            def v_chunks(b):
                return [lambda b=b, g01=g01, t=t: v_chunk(b, g01, t)
                        for g01 in range(2) for t in range(4)]

            def fin_chunks(b, half):
                return [lambda b=b, half=half, qt=qt: fin_chunk(b, half, qt)
                        for qt in range(NTB)]

            # prologue: V(b0) + first unit's Q/K projection emitted directly
            for f in v_chunks(0):
                f()
            for f in qk_chunks(0, 0):
                f()
            # per-unit feeders: future work drips into the attention kt-loop
            attention(0, 0, qk_chunks(0, 1))
            attention(0, 1, qk_chunks(0, 2))
            attention(0, 2, qk_chunks(0, 3) + v_chunks(1)[:4])
            attention(0, 3, v_chunks(1)[4:] + qk_chunks(1, 0))
            attention(1, 0, qk_chunks(1, 1) + fin_chunks(0, 0))
            attention(1, 1, qk_chunks(1, 2) + fin_chunks(0, 1))
            attention(1, 2, qk_chunks(1, 3) + fin_chunks(1, 0))
            attention(1, 3, [])
            for f in fin_chunks(1, 1):
                f()

    nc.compile()
    return nc


_NC_CACHE = None


def _get_nc():
    global _NC_CACHE
    if _NC_CACHE is None:
        _NC_CACHE = build_nc()
    return _NC_CACHE


def prep_inputs(x, pe0, pe1, pe2, mask, W_qkv, b_qkv):
    """Host-side layout prep + per-core sharding. Returns list of in_maps."""
    x = np.asarray(x, dtype=np.float32)
    pe0 = np.asarray(pe0, dtype=np.float32).reshape(T, D)
    pe1 = np.asarray(pe1, dtype=np.float32).reshape(T, D)
    pe2 = np.asarray(pe2, dtype=np.float32).reshape(T, D)
    mask = np.asarray(mask).astype(bool).reshape(B, T, T)
    W_qkv = np.asarray(W_qkv, dtype=np.float32)
    b_qkv = np.asarray(b_qkv, dtype=np.float32)

    wT = np.ascontiguousarray(W_qkv.T).astype(np.float16)   # [C, 3C]
    brow_v = np.ascontiguousarray(
        b_qkv[None, 2 * C:3 * C]).astype(np.float16)        # [1, C]
    ones_row = np.ones((1, 128), dtype=np.float16)
    swp = np.kron(np.eye(64, dtype=np.float32),
                  np.array([[0, 1], [1, 0]], np.float32)
                  ).astype(np.float16)                      # [128,128] pair swap
    bias_qk = np.ascontiguousarray(
        b_qkv[:2 * C].reshape(8, 128).T)                    # [128, 8]

    # rope tables in [d, t] layout, duplicated across the two heads of a
    # partition chunk. Bq/Bk carry the rotate_half sign on the OUTPUT index:
    # out[2i] += -in[2i+1]*pe1, out[2i+1] += +in[2i]*pe1.
    sign = np.ones(D, dtype=np.float32)
    sign[0::2] = -1.0
    Aq = (pe0 * pe2).T                                      # [D, T]
    Bq = (pe1 * pe2).T * sign[:, None]
    Ak = (pe0 / pe2).T
    Bk = (pe1 / pe2).T * sign[:, None]
    ropeT = np.stack(
        [np.tile(t, (2, 1)) for t in (Aq, Bq, Ak, Bk)]
    ).astype(np.float16)                                    # [4, 128, T]

    notmask = (~mask).astype(ml_dtypes.bfloat16)            # [B,T,T] {0,1}
    in_maps = []
    for c in range(N_CORES):
        bs = slice(c * BPC, (c + 1) * BPC)
        xc = np.ascontiguousarray(
            x[bs].reshape(TOK, C).T).astype(np.float16)     # [C, TOK]
        nmT = np.ascontiguousarray(
            notmask[bs].transpose(0, 2, 1))                 # [BPC, T(kv), T(q)]
        in_maps.append(dict(
            xT=xc, wT=wT, brow_v=brow_v, ones_row=ones_row,
            swp=swp, bias_qk=bias_qk, ropeT=ropeT, nmT=nmT,
        ))
    return in_maps


def assemble_output(results):
    out = np.empty((B, T, C), dtype=np.float32)
    for c in range(N_CORES):
        out[c * BPC:(c + 1) * BPC] = results[c]["y"].reshape(BPC, T, C)
    return out


def kernel(x, pe0, pe1, pe2, mask, W_qkv, b_qkv):
    nc = _get_nc()
    in_maps = prep_inputs(x, pe0, pe1, pe2, mask, W_qkv, b_qkv)
    res = run_bass_kernel_spmd(nc, in_maps, core_ids=list(range(N_CORES)))
    return assemble_output(res.results)


# revision 3
# speedup vs baseline: 1.0392x; 1.0392x over previous
"""Trainium2 Bass kernel for masked multi-head attention with a rope-like
positional transform (nn_Attention_43937515438607) — v3.

Architecture (per core, 2 batches):
  - Q,K projected in TRANSPOSED [d, tok] layout (lhsT = W chunk, rhs = x^T);
    V in natural layout with a ones-column for the softmax denominator.
  - rope: bias added in the PSUM->SBUF staging copy (tensor_scalar with a
    per-partition bias AP); pair-swap via one PE matmul against a constant
    128x128 pair-permutation; two fp16 multiplies + add on VectorE.
  - scores: two K=64 matmuls per kv tile into disjoint PE row groups
    (auto tile_position from base partitions 0/64) -> run CONCURRENTLY.
  - softmax: exp on ScalarE straight out of PSUM (no max subtraction,
    validated range), alpha in bf16; {0,1} mask multiply on VectorE.
  - AV: V_ext (64 v rows + ones row) as stationary, alpha moving,
    accumulated over kv tiles; output normalized after a PE transpose.
  - emission is software-pipelined at CHUNK granularity: projection and
    output-normalization chunks for future units are fed one-at-a-time
    into the attention kt-loop, so no engine FIFO gets a long run of
    potentially-blocked instructions (head-of-line blocking).

Measured per-op HW costs that shaped this: exp[128,1024] 1218ns;
TT[128,512]16b 241ns; PSUM-source DVE ops ~630-980ns; S-pair 225ns
(concurrent); AV mm 222ns; GpSimd TT 1203ns + it locks the shared SBUF
port pair against DVE tensor_tensor ops, so GpSimd gets no elementwise
work; scalar_tensor_tensor and stream_shuffle are ~776ns (avoided).
"""

import sys

try:
    import concourse  # noqa: F401
except ImportError:  # pragma: no cover
    sys.path.insert(0, "/opt/trn_rl_repo")

import numpy as np
import ml_dtypes

from concourse import bass, mybir, tile, bacc
from concourse.bass_utils import run_bass_kernel_spmd
from concourse.masks import make_identity

B, T, C = 16, 1024, 512
NH = 8
D = C // NH
TP = float((2.0 * D) ** 0.5)
N_CORES = 8
BPC = B // N_CORES            # batches per core = 2
TOK = BPC * T                 # tokens per core  = 2048
NTT = TOK // 128              # token tiles per core = 16
NTB = T // 128                # token tiles per batch = 8
NHP = NH // 2                 # head pairs = 4
QC = 512                      # q chunk (PSUM bank) per attention unit
NQC = T // QC                 # q chunks per batch = 2
NG = TOK // 512               # 512-token groups per core = 4
VW = 66 * NH + 32             # V_ext row width = 560

F32 = mybir.dt.float32
F32R = mybir.dt.float32r
F16 = mybir.dt.float16
BF16 = mybir.dt.bfloat16
MULT = mybir.AluOpType.mult
ADD = mybir.AluOpType.add


def build_nc(niter=1):
    nc = bacc.Bacc("TRN2", target_bir_lowering=False, debug=False)

    xT_d = nc.dram_tensor("xT", [C, TOK], F16, kind="ExternalInput")
    wT_d = nc.dram_tensor("wT", [C, 3 * C], F16, kind="ExternalInput")
    brow_d = nc.dram_tensor("brow_v", [1, C], F16, kind="ExternalInput")
    ones_d = nc.dram_tensor("ones_row", [1, 128], F16, kind="ExternalInput")
    swp_d = nc.dram_tensor("swp", [128, 128], F16, kind="ExternalInput")
    bqk_d = nc.dram_tensor("bias_qk", [128, 8], F32, kind="ExternalInput")
    rope_d = nc.dram_tensor("ropeT", [4, 128, T], F16, kind="ExternalInput")
    nmT_d = nc.dram_tensor("nmT", [BPC, T, T], BF16, kind="ExternalInput")
    y_d = nc.dram_tensor("y", [TOK, C], F32, kind="ExternalOutput")

    with tile.TileContext(nc) as tc:
        import contextlib
        loop_cm = (tc.For_i(0, niter, 1, staggered_reset=True,
                            hint_engines=(mybir.EngineType.PE,
                                          mybir.EngineType.DVE,
                                          mybir.EngineType.Activation,
                                          mybir.EngineType.SP))
                   if niter > 1 else contextlib.nullcontext())
        ctx = contextlib.ExitStack()
        with loop_cm, ctx:
            persist = ctx.enter_context(tc.tile_pool(name="persist", bufs=1))
            V_sb = persist.tile([128, NTT, VW], BF16)
            QT = [persist.tile([128, NHP, T], F16, tag=f"QT{b}", name=f"QT{b}")
                  for b in range(BPC)]
            KT = [persist.tile([128, NHP, T], F16, tag=f"KT{b}", name=f"KT{b}")
                  for b in range(BPC)]
            OT = [persist.tile([96, NH, T], F32R, tag=f"OT{b}", name=f"OT{b}")
                  for b in range(BPC)]
            mT = [persist.tile([128, NTB, T], BF16, tag=f"mT{b}", name=f"mT{b}")
                  for b in range(BPC)]
            wt = persist.tile([128, 4, 3 * C], F16)
            xg = persist.tile([128, NG, 4, 512], F16)     # [ci, group, ko, tok]
            rp = persist.tile([128, 4, T], F16)           # Aq, Bq, Ak, Bk
            swp = persist.tile([128, 128], F16)
            bqk = persist.tile([128, 8], F32)
            ones1 = persist.tile([1, 128], F16)
            brow = persist.tile([1, C], F16)
            id_tmp = persist.tile([128, 128], F32)
            id_f32 = persist.tile([128, 128], F32R)

            make_identity(nc, id_tmp[:])
            nc.vector.tensor_copy(id_f32[:], id_tmp[:])
            nc.gpsimd.memset(V_sb[:], 0.0)
            nc.vector.memset(V_sb[:, :, 64::66], 1.0)

            # ---------- input DMAs ----------
            for ko in range(4):
                nc.sync.dma_start(wt[:, ko, :], wT_d[ko * 128:(ko + 1) * 128, :])
            nc.sync.dma_start(ones1[:], ones_d[:])
            nc.sync.dma_start(brow[:], brow_d[:])
            nc.sync.dma_start(swp[:], swp_d[:])
            nc.sync.dma_start(bqk[:], bqk_d[:])
            nc.sync.dma_start(rp[:], rope_d.rearrange("f p t -> p f t"))
            for g in range(NG):
                nc.sync.dma_start(
                    xg[:, g],
                    xT_d[:, g * 512:(g + 1) * 512].rearrange(
                        "(ko p) t -> p ko t", p=128))
            for b in range(BPC):
                for kg in range(4):
                    nc.sync.dma_start(
                        mT[b][:, kg * 2:(kg + 1) * 2, :],
                        nmT_d[b][kg * 256:(kg + 1) * 256, :].rearrange(
                            "(kt p) q -> p kt q", p=128))

            pp = ctx.enter_context(tc.tile_pool(name="pp", bufs=2, space="PSUM"))
            s_ps = ctx.enter_context(tc.tile_pool(name="s_ps", bufs=2, space="PSUM"))
            o_ps = ctx.enter_context(tc.tile_pool(name="o_ps", bufs=1, space="PSUM"))
            qsb_pool = ctx.enter_context(tc.tile_pool(name="qsb", bufs=2))
            t_pool = ctx.enter_context(tc.tile_pool(name="tpl", bufs=2))
            alpha_pool = ctx.enter_context(tc.tile_pool(name="alpha", bufs=5))
            fin_sb = ctx.enter_context(tc.tile_pool(name="fin_sb", bufs=3))

            def v_chunk(b, g01, t):
                g = 2 * b + g01
                tt = 4 * g + t
                ps = pp.tile([128, 512], F32, tag="pp", name="vps")
                for ko in range(4):
                    nc.tensor.matmul(
                        ps[:], xg[:, g, ko, t * 128:(t + 1) * 128],
                        wt[:, ko, 2 * C:3 * C], start=(ko == 0), stop=False)
                nc.tensor.matmul(ps[:], ones1[:], brow[:],
                                 start=False, stop=True)
                vdst = V_sb[:, tt, :528].rearrange(
                    "p (h e) -> p h e", h=NH)[:, :, :D]
                eng = nc.scalar if (tt % 2 == 0) else nc.vector
                eng_copy = (nc.scalar.copy if (tt % 2 == 0)
                            else nc.vector.tensor_copy)
                eng_copy(vdst, ps[:].rearrange("p (h d) -> p h d", h=NH))

            def qk_chunk(b, hp, fc, g01):
                col0 = fc * C + hp * 128
                dstt = (QT if fc == 0 else KT)[b]
                g = 2 * b + g01
                tsl = slice(g01 * 512, (g01 + 1) * 512)
                ps = pp.tile([128, 512], F32, tag="pp", name="qkps")
                for ko in range(4):
                    nc.tensor.matmul(
                        ps[:], wt[:, ko, col0:col0 + 128],
                        xg[:, g, ko, :], start=(ko == 0), stop=(ko == 3))
                qsb = qsb_pool.tile([128, 512], F16, tag="qsb")
                nc.vector.tensor_scalar(
                    qsb[:], ps[:], bqk[:, 4 * fc + hp:4 * fc + hp + 1],
                    None, ADD)
                sw = pp.tile([128, 512], F32, tag="pp", name="swps")
                nc.tensor.matmul(sw[:], swp[:], qsb[:], start=True, stop=True)
                t1 = t_pool.tile([128, 512], F16, tag="t1")
                nc.vector.tensor_tensor(t1[:], qsb[:], rp[:, 2 * fc, tsl], MULT)
                t2 = t_pool.tile([128, 512], F16, tag="t2")
                nc.vector.tensor_tensor(t2[:], sw[:], rp[:, 2 * fc + 1, tsl],
                                        MULT)
                nc.vector.tensor_tensor(dstt[:, hp, tsl], t1[:], t2[:], ADD)

            def fin_chunk(b, half, qt):
                out_sb = fin_sb.tile([128, C // 2], F32, tag="out")
                fp = pp.tile([128, 4 * 96], F32R, tag="pp", name="fin")
                for hh in range(4):
                    h = half * 4 + hh
                    nc.tensor.matmul(
                        fp[:, hh * 96:(hh + 1) * 96],
                        OT[b][:, h, qt * 128:(qt + 1) * 128],
                        id_f32[0:96, 0:96], is_transpose=True)
                rc = fin_sb.tile([128, 4], F32, tag="rc")
                nc.vector.reciprocal(rc[:], fp[:, 64::96])
                nc.vector.tensor_tensor(
                    out_sb[:].rearrange("p (h d) -> p h d", h=4),
                    fp[:].rearrange("p (h e) -> p h e", e=96)[:, :, :D],
                    rc[:][:, :, None].to_broadcast([128, 4, D]), MULT)
                row = b * T + qt * 128
                nc.sync.dma_start(
                    y_d[row:row + 128, half * 256:(half + 1) * 256], out_sb[:])

            def attention(b, hp, feed):
                hA, hB = 2 * hp, 2 * hp + 1
                for qc in range(NQC):
                    qsl = slice(qc * QC, (qc + 1) * QC)
                    oo = o_ps.tile([96, 2 * QC], F32, tag="oo")
                    oA, oB = oo[:, 0:QC], oo[:, QC:2 * QC]

                    def emit_av(al, kt):
                        vbase = b * NTB + kt
                        nc.tensor.matmul(
                            oA, V_sb[:, vbase, hA * 66:hA * 66 + 96],
                            al[:, 0:QC],
                            start=(kt == 0), stop=(kt == NTB - 1))
                        nc.tensor.matmul(
                            oB, V_sb[:, vbase, hB * 66:hB * 66 + 96],
                            al[:, QC:2 * QC],
                            start=(kt == 0), stop=(kt == NTB - 1))

                    pend = []
                    for kt in range(NTB):
                        sp = s_ps.tile([128, 2 * QC], F32, tag="s")
                        nc.tensor.matmul(
                            sp[:, 0:QC],
                            KT[b][0:64, hp, kt * 128:(kt + 1) * 128],
                            QT[b][0:64, hp, qsl], start=True, stop=True)
                        nc.tensor.matmul(
                            sp[:, QC:2 * QC],
                            KT[b][64:128, hp, kt * 128:(kt + 1) * 128],
                            QT[b][64:128, hp, qsl], start=True, stop=True)
                        al = alpha_pool.tile([128, 2 * QC], BF16, tag="al")
                        nc.scalar.activation(
                            al[:], sp[:],
                            mybir.ActivationFunctionType.Exp, scale=1.0 / TP)
                        msl = mT[b][:, kt, qsl]
                        nc.vector.tensor_tensor(
                            al[:, 0:QC], al[:, 0:QC], msl, MULT)
                        nc.vector.tensor_tensor(
                            al[:, QC:2 * QC], al[:, QC:2 * QC], msl, MULT)
                        pend.append((al, kt))
                        if len(pend) > 2:
                            emit_av(*pend.pop(0))
                        if feed:
                            feed.pop(0)()
                    for p_ in pend:
                        emit_av(*p_)
                    ot_copy = (nc.scalar.copy if (hp + qc) % 2 == 0
                               else nc.vector.tensor_copy)
                    ot_copy(
                        OT[b][:, hA:hB + 1, qsl], oo[:].rearrange(
                            "p (h q) -> p h q", h=2))

            def qk_chunks(b, hp):
                return [lambda b=b, hp=hp, fc=fc, g01=g01:
                        qk_chunk(b, hp, fc, g01)
                        for fc in range(2) for g01 in range(2)]

            def v_chunks(b):
                return [lambda b=b, g01=g01, t=t: v_chunk(b, g01, t)
                        for g01 in range(2) for t in range(4)]

            def fin_chunks(b, half):
                return [lambda b=b, half=half, qt=qt: fin_chunk(b, half, qt)
                        for qt in range(NTB)]

            # prologue: first unit's Q/K projection, then V(b0)
            for f in qk_chunks(0, 0):
                f()
            for f in v_chunks(0):
                f()
            # per-unit feeders: future work drips into the attention kt-loop
            attention(0, 0, qk_chunks(0, 1))
            attention(0, 1, qk_chunks(0, 2))
            attention(0, 2, qk_chunks(0, 3) + v_chunks(1)[:4])
            attention(0, 3, v_chunks(1)[4:] + qk_chunks(1, 0))
            attention(1, 0, qk_chunks(1, 1) + fin_chunks(0, 0))
            attention(1, 1, qk_chunks(1, 2) + fin_chunks(0, 1))
            attention(1, 2, qk_chunks(1, 3) + fin_chunks(1, 0))
            attention(1, 3, [])
            for f in fin_chunks(1, 1):
                f()

    nc.compile()
    return nc


_NC_CACHE = None


def _get_nc():
    global _NC_CACHE
    if _NC_CACHE is None:
        _NC_CACHE = build_nc()
    return _NC_CACHE


def prep_inputs(x, pe0, pe1, pe2, mask, W_qkv, b_qkv):
    """Host-side layout prep + per-core sharding. Returns list of in_maps."""
    x = np.asarray(x, dtype=np.float32)
    pe0 = np.asarray(pe0, dtype=np.float32).reshape(T, D)
    pe1 = np.asarray(pe1, dtype=np.float32).reshape(T, D)
    pe2 = np.asarray(pe2, dtype=np.float32).reshape(T, D)
    mask = np.asarray(mask).astype(bool).reshape(B, T, T)
    W_qkv = np.asarray(W_qkv, dtype=np.float32)
    b_qkv = np.asarray(b_qkv, dtype=np.float32)

    wT = np.ascontiguousarray(W_qkv.T).astype(np.float16)   # [C, 3C]
    brow_v = np.ascontiguousarray(
        b_qkv[None, 2 * C:3 * C]).astype(np.float16)        # [1, C]
    ones_row = np.ones((1, 128), dtype=np.float16)
    swp = np.kron(np.eye(64, dtype=np.float32),
                  np.array([[0, 1], [1, 0]], np.float32)
                  ).astype(np.float16)                      # [128,128] pair swap
    bias_qk = np.ascontiguousarray(
        b_qkv[:2 * C].reshape(8, 128).T)                    # [128, 8]

    # rope tables in [d, t] layout, duplicated across the two heads of a
    # partition chunk. Bq/Bk carry the rotate_half sign on the OUTPUT index:
    # out[2i] += -in[2i+1]*pe1, out[2i+1] += +in[2i]*pe1.
    sign = np.ones(D, dtype=np.float32)
    sign[0::2] = -1.0
    Aq = (pe0 * pe2).T                                      # [D, T]
    Bq = (pe1 * pe2).T * sign[:, None]
    Ak = (pe0 / pe2).T
    Bk = (pe1 / pe2).T * sign[:, None]
    ropeT = np.stack(
        [np.tile(t, (2, 1)) for t in (Aq, Bq, Ak, Bk)]
    ).astype(np.float16)                                    # [4, 128, T]

    notmask = (~mask).astype(ml_dtypes.bfloat16)            # [B,T,T] {0,1}
    in_maps = []
    for c in range(N_CORES):
        bs = slice(c * BPC, (c + 1) * BPC)
        xc = np.ascontiguousarray(
            x[bs].reshape(TOK, C).T).astype(np.float16)     # [C, TOK]
        nmT = np.ascontiguousarray(
            notmask[bs].transpose(0, 2, 1))                 # [BPC, T(kv), T(q)]
        in_maps.append(dict(
            xT=xc, wT=wT, brow_v=brow_v, ones_row=ones_row,
            swp=swp, bias_qk=bias_qk, ropeT=ropeT, nmT=nmT,
        ))
    return in_maps


def assemble_output(results):
    out = np.empty((B, T, C), dtype=np.float32)
    for c in range(N_CORES):
        out[c * BPC:(c + 1) * BPC] = results[c]["y"].reshape(BPC, T, C)
    return out


def kernel(x, pe0, pe1, pe2, mask, W_qkv, b_qkv):
    nc = _get_nc()
    in_maps = prep_inputs(x, pe0, pe1, pe2, mask, W_qkv, b_qkv)
    res = run_bass_kernel_spmd(nc, in_maps, core_ids=list(range(N_CORES)))
    return assemble_output(res.results)
